# revision 40
# baseline (speedup 1.0000x reference)
"""RNN-T joint network (dense MLP) Trainium2 Bass kernel.

Math (per batch row n):
    h = relu(f @ W1t.T + g @ W1p.T + b1t + b1p)    # [N, 512]
    y = h @ W2.T + b2                              # [N, 29]

Strategy: data-parallel over batch N=32768 across 8 NeuronCores (4096
rows/core); weights replicated.  Both layers run on the PE array in fp8
(e4m3) DoubleRow perf mode at 0.5 cycles/row -- 2x the bf16/fp32r rate.
Accuracy is recovered with a mixed-precision decomposition computed on
the host:
    x = (xhi + xlo)/sx        xhi, xlo = e4m3 hi/lo split of sx*x
    w = (whi + wlo)/sw        whi, wlo = e4m3 hi/lo split of sw*w
    x@w ~= [(xhi+xlo)@whi + xhi@wlo] / (sx*sw)
Layer 1 per j-tile is 12 DoubleRow tiles: 9 k-slices pair (xhi, xlo)
against slot-broadcast whi (a stride-0 AP, so whi is stored/DMAed
once); the 2 lowest-weight-variance f-slices drop xlo and share one
tile; 2 correction tiles apply xhi@wlo where weight-quantization
variance is largest (the g/PRED block has 1.8x larger weights -- a
k-slice permutation puts those slices first so correction pairs are
AP-contiguous).  Layer 2 uses the same scheme on h (hi/lo split via
DVE copy/sub) with W2 hi/lo, except the last chunk, which stays bf16
to keep the drain tail short.  Realized max-rel-err 1.78e-2 (< 2e-2
gate, deterministic inputs).  All scales are powers of two, folded
into b1/W2/the output activation via positive homogeneity of relu.

Schedule: the layer-2 block of chunk c-1 is emitted inside chunk c to
hide the h-activation latency; chunk 0 runs all whi blocks (j0/j1
k-interleaved to match DMA delivery) before the wlo correction blocks
so the wlo DMA can land late; x-chunk DMA pieces interleave with weight
pieces on the SP ring to keep the PE fed during the pipeline fill;
small consts ride the Pool/SWDGE ring so they cost no HWDGE slots.
"""

import numpy as np
import ml_dtypes

import concourse.bacc as bacc
import concourse.bass as bass  # noqa: F401
import concourse.mybir as mybir
from concourse import tile
from concourse.bass_utils import run_bass_kernel_spmd

TRANS_H, PRED_H, JOINT_H, NUM_LABELS = 1024, 320, 512, 29
BATCH = 32768
N_CORES = 8
N_PER_CORE = BATCH // N_CORES          # 4096
K_TOTAL = TRANS_H + PRED_H             # 1344
K_PAD = 1408                           # 11 * 128
KT = K_PAD // 128                      # 11 k-slices
T3 = 2                                 # wlo-correction DoubleRow tiles
J_TILES = JOINT_H // 128               # 4
N_CHUNK = 512                          # PSUM-bank limit (2KB fp32)
N_CHUNKS = N_PER_CORE // N_CHUNK       # 8

# k-slice order in SBUF: g-block slices (8,9,10) first so the T3
# correction tiles cover pairs (8,9),(10,0),(1,2) contiguously.
PERM = [8, 9, 10, 0, 1, 2, 3, 4, 5, 6, 7]

SX = 16.0      # x fp8 scale
SW = 2048.0    # w fp8 scale
SXW = SX * SW  # 32768
SH = 2.0 ** -10  # psum -> hsc frame (hsc = 32*h)
S2 = 512.0       # w2 fp8 frame: (32h)@(512 W2) = 16384 y
LAB_PAD = 32     # dual-fp8 ldweights needs a multiple-of-4 column count
KT_HL = 9        # k-slices with full hi/lo x pairs (9,10 are hi-only)

F32 = mybir.dt.float32
BF16 = mybir.dt.bfloat16
F8 = mybir.dt.float8e4
DR = mybir.MatmulPerfMode.DoubleRow
E4 = ml_dtypes.float8_e4m3
BF = ml_dtypes.bfloat16

_NC_CACHE = {}


def _build_bass():
    """Build the single-core Bass program (same NEFF runs SPMD on 8 cores)."""
    nc = bacc.Bacc(None)

    xq = nc.dram_tensor("xq", [128, N_CHUNKS, KT, 2, N_CHUNK], F8,
                        kind="ExternalInput")
    wq = nc.dram_tensor("wq", [128, J_TILES, KT, 128], F8,
                        kind="ExternalInput")
    wlo = nc.dram_tensor("wlo", [128, T3, 2, JOINT_H], F8,
                         kind="ExternalInput")
    b1s = nc.dram_tensor("b1s", [128, J_TILES], F32, kind="ExternalInput")
    w2b = nc.dram_tensor("w2b", [128, J_TILES, NUM_LABELS], BF16,
                         kind="ExternalInput")
    w2q = nc.dram_tensor("w2q", [128, J_TILES, 2, LAB_PAD], F8,
                         kind="ExternalInput")
    w2l = nc.dram_tensor("w2l", [128, 2, 2, LAB_PAD], F8,
                         kind="ExternalInput")
    b2c = nc.dram_tensor("b2c", [NUM_LABELS, 1], F32, kind="ExternalInput")
    yT = nc.dram_tensor("yT", [NUM_LABELS, N_PER_CORE], F32,
                        kind="ExternalOutput")

    K_SPLITS = [(0, 6), (6, KT)]
    K_SPLITS_FILL = [(0, 3), (3, 6), (6, 9), (9, KT)]

    with tile.TileContext(nc) as tc:
        with (
            tc.tile_pool(name="consts", bufs=1) as consts,
            tc.tile_pool(name="xpool", bufs=4) as xpool,
            tc.tile_pool(name="hpool", bufs=2) as hpool,
            tc.tile_pool(name="opool", bufs=2) as opool,
            tc.tile_pool(name="psum_h", bufs=6, space="PSUM") as psum_h,
            tc.tile_pool(name="psum_y", bufs=2, space="PSUM") as psum_y,
        ):
            # ---- replicated constants; explicit SP-ring order interleaves
            # weight pieces with the first x chunks so the PE never starves
            # small consts ride the Pool/SWDGE ring: no HWDGE slot consumed
            wq_sb = consts.tile([128, J_TILES, KT, 128], F8, name="wq_sb",
                                tag="wq")
            wlo_sb = consts.tile([128, T3, 2, JOINT_H], F8, name="wlo_sb",
                                 tag="wlo")
            b1_sb = consts.tile([128, J_TILES], F32, name="b1_sb", tag="b1")
            nc.gpsimd.dma_start(out=b1_sb, in_=b1s[:, :])
            w2_sb = consts.tile([128, J_TILES, NUM_LABELS], BF16,
                                name="w2_sb", tag="w2")
            nc.gpsimd.dma_start(out=w2_sb, in_=w2b[:, :, :])
            b2_sb = consts.tile([NUM_LABELS, 1], F32, name="b2_sb", tag="b2")
            nc.gpsimd.dma_start(out=b2_sb, in_=b2c[:, :])
            w2q_sb = consts.tile([128, J_TILES, 2, LAB_PAD], F8,
                                 name="w2q_sb", tag="w2q")
            nc.gpsimd.dma_start(out=w2q_sb, in_=w2q[:, :, :, :])
            w2l_sb = consts.tile([128, 2, 2, LAB_PAD], F8,
                                 name="w2l_sb", tag="w2l")
            nc.gpsimd.dma_start(out=w2l_sb, in_=w2l[:, :, :, :])

            def l1_b_block(x_sb, j, n_sl, ph):
                """Layer-1 (xhi+xlo)@whi part.

                k-slices 0..8 pair (xhi, xlo) against slot-broadcast whi;
                slices 9,10 (lowest-variance f slices) drop xlo and pair
                their two xhi slices into a single DoubleRow tile.
                """
                for k in range(KT_HL):
                    nc.tensor.matmul(
                        ph,
                        lhsT=wq_sb[:, j, k].unsqueeze(1)
                                           .broadcast_to([128, 2, 128]),
                        rhs=x_sb[:, k, :, n_sl],
                        start=(k == 0),
                        stop=False,
                        perf_mode=DR,
                    )
                nc.tensor.matmul(
                    ph,
                    lhsT=wq_sb[:, j, KT_HL:KT, :],
                    rhs=x_sb[:, KT_HL:KT, 0, n_sl],
                    start=False,
                    stop=False,
                    perf_mode=DR,
                )

            def l1_t3_block(x_sb, j, n_sl, ph):
                """Layer-1 xhi@wlo correction: 3 DR tiles, stops the group."""
                jsl = slice(j * 128, (j + 1) * 128)
                for t in range(T3):
                    nc.tensor.matmul(
                        ph,
                        lhsT=wlo_sb[:, t, :, jsl],
                        rhs=x_sb[:, 2 * t:2 * t + 2, 0, n_sl],
                        start=False,
                        stop=(t == T3 - 1),
                        perf_mode=DR,
                    )

            def l2_block(hx, n0, width, final=False):
                """Layer 2: y[:, n0:n0+width] + bias + DMA out.

                hx is either an fp8 hi/lo tile [128, 4, 2, width] (DoubleRow
                path, 6 matmuls) or a list of 4 bf16 h tiles (last chunk).
                """
                fp8_path = not isinstance(hx, list)
                py = psum_y.tile([LAB_PAD if fp8_path else NUM_LABELS, width],
                                 F32, name="py", tag="py")
                if not fp8_path:
                    for j in range(J_TILES):
                        nc.tensor.matmul(
                            py,
                            lhsT=w2_sb[:, j],
                            rhs=hx[j],
                            start=(j == 0),
                            stop=(j == J_TILES - 1),
                        )
                    yscale = 1.0
                else:
                    for js in range(J_TILES):
                        nc.tensor.matmul(
                            py,
                            lhsT=w2q_sb[:, js],
                            rhs=hx[:, js],
                            start=(js == 0),
                            stop=False,
                            perf_mode=DR,
                        )
                    for t2 in range(2):
                        nc.tensor.matmul(
                            py,
                            lhsT=w2l_sb[:, t2],
                            rhs=hx[:, 2 * t2:2 * t2 + 2, 0, :],
                            start=False,
                            stop=(t2 == 1),
                            perf_mode=DR,
                        )
                    yscale = 1.0 / 16384.0
                y_sb = opool.tile([NUM_LABELS, width], F32, name="y_sb",
                                  tag="y")
                nc.scalar.activation(
                    y_sb, py[0:NUM_LABELS, :],
                    mybir.ActivationFunctionType.Identity,
                    bias=b2_sb, scale=yscale,
                )
                eng = nc.sync if final else nc.scalar
                eng.dma_start(out=yT[:, n0:n0 + width], in_=y_sb)

            # ---- chunk 0: interleave weight DMAs with x pieces; run all
            # B-blocks before the correction blocks so wlo can arrive late
            x0_sb = xpool.tile([128, KT, 2, N_CHUNK], F8, name="x_sb",
                               tag="x")
            nc.sync.dma_start(out=wq_sb[:, 0], in_=wq[:, 0])
            nc.sync.dma_start(out=x0_sb[:, 0:2], in_=xq[:, 0, 0:2])
            nc.sync.dma_start(out=wq_sb[:, 1], in_=wq[:, 1])
            nc.sync.dma_start(out=x0_sb[:, 2:6], in_=xq[:, 0, 2:6])
            nc.sync.dma_start(out=x0_sb[:, 6:KT_HL], in_=xq[:, 0, 6:KT_HL])
            nc.sync.dma_start(out=x0_sb[:, KT_HL:KT, 0:1],
                              in_=xq[:, 0, KT_HL:KT, 0:1])
            nc.sync.dma_start(out=wq_sb[:, 2:4], in_=wq[:, 2:4])
            nc.sync.dma_start(out=wlo_sb, in_=wlo[:, :, :, :])

            # ---- main loop; sub-chunks: 7 full 512s + 2 halves of 256 ----
            # (c, n_sl within x tile); chunk 7's x tile serves both halves
            pending = None  # (h_tiles, n0, width) awaiting layer 2
            x_sb = x0_sb
            full = slice(0, N_CHUNK)
            subchunks = [(c, full) for c in range(N_CHUNKS)]
            for c, n_sl in subchunks:
                if n_sl.start == 0 and c > 0:
                    x_sb = xpool.tile([128, KT, 2, N_CHUNK], F8, name="x_sb",
                                      tag="x")
                    splits = ([(0, 4), (4, 8), (8, KT_HL)] if c == 1
                              else [(0, 6), (6, KT_HL)])
                    for (ka, kb) in splits:
                        nc.sync.dma_start(out=x_sb[:, ka:kb],
                                          in_=xq[:, c, ka:kb])
                    nc.sync.dma_start(out=x_sb[:, KT_HL:KT, 0:1],
                                      in_=xq[:, c, KT_HL:KT, 0:1])
                width = n_sl.stop - n_sl.start
                h_tiles = []
                if c == 0:
                    # deferred group structure: 4 open psum groups.  j0/j1
                    # consume x pieces k-interleaved (two matmuls per slice
                    # matches the DMA delivery rate); j2/j3 run after their
                    # weights land, the wlo correction blocks last.
                    phs = [psum_h.tile([128, width], F32, name=f"ph_{j}",
                                       tag="ph") for j in range(J_TILES)]
                    for k in range(KT_HL):
                        for j in (0, 1):
                            nc.tensor.matmul(
                                phs[j],
                                lhsT=wq_sb[:, j, k].unsqueeze(1)
                                                   .broadcast_to([128, 2,
                                                                  128]),
                                rhs=x_sb[:, k, :, n_sl],
                                start=(k == 0),
                                stop=False,
                                perf_mode=DR,
                            )
                    for j in (0, 1):
                        nc.tensor.matmul(
                            phs[j],
                            lhsT=wq_sb[:, j, KT_HL:KT, :],
                            rhs=x_sb[:, KT_HL:KT, 0, n_sl],
                            start=False,
                            stop=False,
                            perf_mode=DR,
                        )
                    for j in (2, 3):
                        l1_b_block(x_sb, j, n_sl, phs[j])
                    h8 = hpool.tile([128, J_TILES, 2, width], F8,
                                    name="h8", tag="h8")
                    for j in range(J_TILES):
                        l1_t3_block(x_sb, j, n_sl, phs[j])
                        h_sb = hpool.tile([128, width], BF16, name=f"h_{j}",
                                          tag=f"h_{j}")
                        nc.scalar.activation(
                            h_sb, phs[j], mybir.ActivationFunctionType.Relu,
                            bias=b1_sb[:, j:j + 1], scale=SH,
                        )
                        nc.vector.tensor_copy(h8[:, j, 0], h_sb)
                        nc.vector.tensor_sub(h8[:, j, 1], h_sb, h8[:, j, 0])
                        h_tiles.append(h_sb)
                else:
                    last = (c == N_CHUNKS - 1)
                    if not last:
                        h8 = hpool.tile([128, J_TILES, 2, width], F8,
                                        name="h8", tag="h8")
                    for j in range(J_TILES):
                        ph = psum_h.tile([128, width], F32, name=f"ph_{j}",
                                         tag="ph")
                        l1_b_block(x_sb, j, n_sl, ph)
                        l1_t3_block(x_sb, j, n_sl, ph)
                        if j == 1 and pending is not None:
                            l2_block(*pending)
                            pending = None
                        h_sb = hpool.tile([128, width], BF16, name=f"h_{j}",
                                          tag=f"h_{j}")
                        nc.scalar.activation(
                            h_sb, ph, mybir.ActivationFunctionType.Relu,
                            bias=b1_sb[:, j:j + 1], scale=SH,
                        )
                        if not last:
                            nc.vector.tensor_copy(h8[:, j, 0], h_sb)
                            nc.vector.tensor_sub(h8[:, j, 1], h_sb,
                                                 h8[:, j, 0])
                        h_tiles.append(h_sb)
                hx = h_tiles if c == N_CHUNKS - 1 else h8
                pending = (hx, c * N_CHUNK + n_sl.start, width)
            l2_block(*pending, final=True)

    nc.finalize()
    return nc


def _get_nc():
    if "nc" not in _NC_CACHE:
        _NC_CACHE["nc"] = _build_bass()
    return _NC_CACHE["nc"]


def _prep_in_maps(f, g, W1t, b1t, W1p, b1p, W2, b2):
    f2 = np.asarray(f, np.float32).reshape(BATCH, TRANS_H)
    g2 = np.asarray(g, np.float32).reshape(BATCH, PRED_H)

    # ---- weights: fp8 hi/lo split at scale SW, k-slices permuted ----
    ws = np.zeros((K_PAD, JOINT_H), np.float32)
    ws[:TRANS_H] = np.asarray(W1t, np.float32).T * SW
    ws[TRANS_H:K_TOTAL] = np.asarray(W1p, np.float32).T * SW
    whi = ws.astype(E4)
    wlo8 = (ws - whi.astype(np.float32)).astype(E4)

    # wq[p, jb, kk, jc] = whi[PERM[kk]*128+p, jb*128+jc]
    whi_s = whi.reshape(KT, 128, J_TILES, 128)[PERM]      # [kk, p, jb, jc]
    wq = np.ascontiguousarray(whi_s.transpose(1, 2, 0, 3))
    # correction tiles cover permuted slice positions (0,1),(2,3),(4,5)
    wlo_s = wlo8.reshape(KT, 128, JOINT_H)[PERM[:2 * T3]]
    wlo_t = np.ascontiguousarray(
        wlo_s.reshape(T3, 2, 128, JOINT_H).transpose(2, 0, 1, 3)
    )

    b1 = (np.asarray(b1t, np.float32) + np.asarray(b1p, np.float32)) * 32.0
    b1s = np.ascontiguousarray(b1.reshape(J_TILES, 128).T)
    w2t = np.asarray(W2, np.float32).T                    # [512, 29]
    w2b = np.ascontiguousarray(
        (w2t / 32.0).reshape(J_TILES, 128, NUM_LABELS).transpose(1, 0, 2)
    ).astype(BF)
    b2c = np.asarray(b2, np.float32).reshape(NUM_LABELS, 1)
    w2src = np.zeros((JOINT_H, LAB_PAD), np.float32)
    w2src[:, :NUM_LABELS] = w2t * S2
    w2hi = w2src.astype(E4)
    w2lo = (w2src - w2hi.astype(np.float32)).astype(E4)
    w2q = np.empty((128, J_TILES, 2, LAB_PAD), E4)
    w2q[:, :, 0] = w2hi.reshape(J_TILES, 128, LAB_PAD).transpose(1, 0, 2)
    w2q[:, :, 1] = w2q[:, :, 0]
    w2l = np.ascontiguousarray(
        w2lo.reshape(2, 2, 128, LAB_PAD).transpose(2, 0, 1, 3)
    )

    # ---- activations: fp8 hi/lo split at scale SX ----
    in_maps = []
    for core in range(N_CORES):
        sl = slice(core * N_PER_CORE, (core + 1) * N_PER_CORE)
        xs = np.zeros((N_PER_CORE, K_PAD), np.float32)
        xs[:, :TRANS_H] = f2[sl] * SX
        xs[:, TRANS_H:K_TOTAL] = g2[sl] * SX
        xhi = xs.astype(E4)
        xlo = (xs - xhi.astype(np.float32)).astype(E4)
        # [n, k] -> [p, c, kk, slot, n']  with row n = c*512+n', col= k*128+p
        xq = np.empty((128, N_CHUNKS, KT, 2, N_CHUNK), E4)
        xq[:, :, :, 0] = (xhi.reshape(N_CHUNKS, N_CHUNK, KT, 128)[:, :, PERM]
                          .transpose(3, 0, 2, 1))
        xq[:, :, :, 1] = (xlo.reshape(N_CHUNKS, N_CHUNK, KT, 128)[:, :, PERM]
                          .transpose(3, 0, 2, 1))
        in_maps.append({"xq": xq, "wq": wq, "wlo": wlo_t, "b1s": b1s,
                        "w2b": w2b, "b2c": b2c, "w2q": w2q, "w2l": w2l})
    return in_maps


def _gather(results):
    y = np.empty((1, BATCH, NUM_LABELS), np.float32)
    for core, r in enumerate(results):
        y[0, core * N_PER_CORE:(core + 1) * N_PER_CORE] = r["yT"].T
    return y


def _run(inputs, trace=False):
    in_maps = _prep_in_maps(
        inputs["f"], inputs["g"], inputs["W1t"], inputs["b1t"],
        inputs["W1p"], inputs["b1p"], inputs["W2"], inputs["b2"],
    )
    res = run_bass_kernel_spmd(
        _get_nc(), in_maps, core_ids=list(range(N_CORES)), trace=trace
    )
    return _gather(res.results), res


def kernel(**inputs) -> np.ndarray:
    out, _ = _run(inputs, trace=False)
    return out


# revision 41
# speedup vs baseline: 1.0684x; 1.0684x over previous
"""RNN-T joint network (dense MLP) Trainium2 Bass kernel.

Math (per batch row n):
    h = relu(f @ W1t.T + g @ W1p.T + b1t + b1p)    # [N, 512]
    y = h @ W2.T + b2                              # [N, 29]

Strategy: data-parallel over batch N=32768 across 8 NeuronCores (4096
rows/core); weights replicated.  Both layers run on the PE array in fp8
(e4m3) DoubleRow perf mode at 0.5 cycles/row -- 2x the bf16/fp32r rate.
Accuracy is recovered with a mixed-precision decomposition computed on
the host:
    x = (xhi + xlo)/sx        xhi, xlo = e4m3 hi/lo split of sx*x
    w = (whi + wlo)/sw        whi, wlo = e4m3 hi/lo split of sw*w
    x@w ~= [(xhi+xlo)@whi + xhi@wlo] / (sx*sw)
Layer 1 per j-tile is 12 DoubleRow tiles: 9 k-slices pair (xhi, xlo)
against slot-broadcast whi (a stride-0 AP, so whi is stored/DMAed
once); the 2 lowest-weight-variance f-slices drop xlo and share one
tile; 2 correction tiles apply xhi@wlo where weight-quantization
variance is largest (the g/PRED block has 1.8x larger weights -- a
k-slice permutation puts those slices first so correction pairs are
AP-contiguous).  Layer 2 uses the same scheme on h (hi/lo split via
DVE copy/sub) with W2 hi/lo, except the last chunk, which stays bf16
to keep the drain tail short.  Realized max-rel-err 1.78e-2 (< 2e-2
gate, deterministic inputs).  All scales are powers of two, folded
into b1/W2/the output activation via positive homogeneity of relu.

Schedule: the layer-2 block of chunk c-1 is emitted inside chunk c to
hide the h-activation latency; chunk 0 runs all whi blocks (j0/j1
k-interleaved to match DMA delivery) before the wlo correction blocks
so the wlo DMA can land late; x-chunk DMA pieces interleave with weight
pieces on the SP ring to keep the PE fed during the pipeline fill;
small consts ride the Pool/SWDGE ring so they cost no HWDGE slots.
"""

import numpy as np
import ml_dtypes

import concourse.bacc as bacc
import concourse.bass as bass  # noqa: F401
import concourse.mybir as mybir
from concourse import tile
from concourse.bass_utils import run_bass_kernel_spmd

TRANS_H, PRED_H, JOINT_H, NUM_LABELS = 1024, 320, 512, 29
BATCH = 32768
N_CORES = 8
N_PER_CORE = BATCH // N_CORES          # 4096
K_TOTAL = TRANS_H + PRED_H             # 1344
K_PAD = 1408                           # 11 * 128
KT = K_PAD // 128                      # 11 k-slices
T3 = 2                                 # wlo-correction DoubleRow tiles
J_TILES = JOINT_H // 128               # 4
N_CHUNK = 512                          # PSUM-bank limit (2KB fp32)
N_CHUNKS = N_PER_CORE // N_CHUNK       # 8

# k-slice order in SBUF: g-block slices (8,9,10) first so the T3
# correction tiles cover pairs (8,9),(10,0),(1,2) contiguously.
PERM = [8, 9, 10, 0, 1, 2, 3, 4, 5, 6, 7]

SX = 16.0      # x fp8 scale
SW = 2048.0    # w fp8 scale
SXW = SX * SW  # 32768
SH = 2.0 ** -10  # psum -> hsc frame (hsc = 32*h)
S2 = 512.0       # w2 fp8 frame: (32h)@(512 W2) = 16384 y
LAB_PAD = 32     # dual-fp8 ldweights needs a multiple-of-4 column count
# B-part tile layout over permuted k-slice positions: 0..5 and 10 keep full
# (xhi, xlo) DoubleRow pairs; positions 6..9 (lowest weight variance) drop
# xlo and pack two xhi slices per tile.
HL_KS = [0, 1, 2, 3, 4, 5, 10]
PAIR_KS = [(6, 8), (8, 10)]

F32 = mybir.dt.float32
BF16 = mybir.dt.bfloat16
F8 = mybir.dt.float8e4
DR = mybir.MatmulPerfMode.DoubleRow
E4 = ml_dtypes.float8_e4m3
BF = ml_dtypes.bfloat16

_NC_CACHE = {}


def _build_bass():
    """Build the single-core Bass program (same NEFF runs SPMD on 8 cores)."""
    nc = bacc.Bacc(None)

    xq = nc.dram_tensor("xq", [128, N_CHUNKS, KT, 2, N_CHUNK], F8,
                        kind="ExternalInput")
    wq = nc.dram_tensor("wq", [128, J_TILES, KT, 128], F8,
                        kind="ExternalInput")
    wlo = nc.dram_tensor("wlo", [128, T3, 2, JOINT_H], F8,
                         kind="ExternalInput")
    b1s = nc.dram_tensor("b1s", [128, J_TILES], F32, kind="ExternalInput")
    w2b = nc.dram_tensor("w2b", [128, J_TILES, NUM_LABELS], BF16,
                         kind="ExternalInput")
    w2q = nc.dram_tensor("w2q", [128, J_TILES, 2, LAB_PAD], F8,
                         kind="ExternalInput")
    w2l = nc.dram_tensor("w2l", [128, 2, 2, LAB_PAD], F8,
                         kind="ExternalInput")
    b2c = nc.dram_tensor("b2c", [NUM_LABELS, 1], F32, kind="ExternalInput")
    yT = nc.dram_tensor("yT", [NUM_LABELS, N_PER_CORE], F32,
                        kind="ExternalOutput")

    K_SPLITS = [(0, 6), (6, KT)]
    K_SPLITS_FILL = [(0, 3), (3, 6), (6, 9), (9, KT)]

    with tile.TileContext(nc) as tc:
        with (
            tc.tile_pool(name="consts", bufs=1) as consts,
            tc.tile_pool(name="xpool", bufs=4) as xpool,
            tc.tile_pool(name="hpool", bufs=2) as hpool,
            tc.tile_pool(name="opool", bufs=2) as opool,
            tc.tile_pool(name="psum_h", bufs=6, space="PSUM") as psum_h,
            tc.tile_pool(name="psum_y", bufs=2, space="PSUM") as psum_y,
        ):
            # ---- replicated constants; explicit SP-ring order interleaves
            # weight pieces with the first x chunks so the PE never starves
            # small consts ride the Pool/SWDGE ring: no HWDGE slot consumed
            wq_sb = consts.tile([128, J_TILES, KT, 128], F8, name="wq_sb",
                                tag="wq")
            wlo_sb = consts.tile([128, T3, 2, JOINT_H], F8, name="wlo_sb",
                                 tag="wlo")
            b1_sb = consts.tile([128, J_TILES], F32, name="b1_sb", tag="b1")
            nc.gpsimd.dma_start(out=b1_sb, in_=b1s[:, :])
            w2_sb = consts.tile([128, J_TILES, NUM_LABELS], BF16,
                                name="w2_sb", tag="w2")
            nc.gpsimd.dma_start(out=w2_sb, in_=w2b[:, :, :])
            b2_sb = consts.tile([NUM_LABELS, 1], F32, name="b2_sb", tag="b2")
            nc.gpsimd.dma_start(out=b2_sb, in_=b2c[:, :])
            w2q_sb = consts.tile([128, J_TILES, 2, LAB_PAD], F8,
                                 name="w2q_sb", tag="w2q")
            nc.gpsimd.dma_start(out=w2q_sb, in_=w2q[:, :, :, :])
            w2l_sb = consts.tile([128, 2, 2, LAB_PAD], F8,
                                 name="w2l_sb", tag="w2l")
            nc.gpsimd.dma_start(out=w2l_sb, in_=w2l[:, :, :, :])

            def b_tile(ph, j, i, x_sb, n_sl):
                """Emit B-part tile i (0..8) for j-tile j into psum ph."""
                if i < len(HL_KS):
                    k = HL_KS[i]
                    nc.tensor.matmul(
                        ph,
                        lhsT=wq_sb[:, j, k].unsqueeze(1)
                                           .broadcast_to([128, 2, 128]),
                        rhs=x_sb[:, k, :, n_sl],
                        start=(i == 0),
                        stop=False,
                        perf_mode=DR,
                    )
                else:
                    ka, kb = PAIR_KS[i - len(HL_KS)]
                    nc.tensor.matmul(
                        ph,
                        lhsT=wq_sb[:, j, ka:kb, :],
                        rhs=x_sb[:, ka:kb, 0, n_sl],
                        start=False,
                        stop=False,
                        perf_mode=DR,
                    )

            N_B_TILES = len(HL_KS) + len(PAIR_KS)

            def l1_b_block(x_sb, j, n_sl, ph):
                """Layer-1 (xhi+xlo)@whi part: 9 DoubleRow tiles."""
                for i in range(N_B_TILES):
                    b_tile(ph, j, i, x_sb, n_sl)

            def l1_t3_block(x_sb, j, n_sl, ph):
                """Layer-1 xhi@wlo correction: 3 DR tiles, stops the group."""
                jsl = slice(j * 128, (j + 1) * 128)
                for t in range(T3):
                    nc.tensor.matmul(
                        ph,
                        lhsT=wlo_sb[:, t, :, jsl],
                        rhs=x_sb[:, 2 * t:2 * t + 2, 0, n_sl],
                        start=False,
                        stop=(t == T3 - 1),
                        perf_mode=DR,
                    )

            def l2_block(hx, n0, width, final=False):
                """Layer 2: y[:, n0:n0+width] + bias + DMA out.

                hx is either an fp8 hi/lo tile [128, 4, 2, width] (DoubleRow
                path, 6 matmuls) or a list of 4 bf16 h tiles (last chunk).
                """
                fp8_path = not isinstance(hx, list)
                py = psum_y.tile([LAB_PAD if fp8_path else NUM_LABELS, width],
                                 F32, name="py", tag="py")
                if not fp8_path:
                    for j in range(J_TILES):
                        nc.tensor.matmul(
                            py,
                            lhsT=w2_sb[:, j],
                            rhs=hx[j],
                            start=(j == 0),
                            stop=(j == J_TILES - 1),
                        )
                    yscale = 1.0
                else:
                    for js in range(J_TILES):
                        nc.tensor.matmul(
                            py,
                            lhsT=w2q_sb[:, js],
                            rhs=hx[:, js],
                            start=(js == 0),
                            stop=False,
                            perf_mode=DR,
                        )
                    for t2 in range(2):
                        nc.tensor.matmul(
                            py,
                            lhsT=w2l_sb[:, t2],
                            rhs=hx[:, 2 * t2:2 * t2 + 2, 0, :],
                            start=False,
                            stop=(t2 == 1),
                            perf_mode=DR,
                        )
                    yscale = 1.0 / 16384.0
                y_sb = opool.tile([NUM_LABELS, width], F32, name="y_sb",
                                  tag="y")
                nc.scalar.activation(
                    y_sb, py[0:NUM_LABELS, :],
                    mybir.ActivationFunctionType.Identity,
                    bias=b2_sb, scale=yscale,
                )
                eng = nc.sync if final else nc.scalar
                eng.dma_start(out=yT[:, n0:n0 + width], in_=y_sb)

            # ---- chunk 0: interleave weight DMAs with x pieces; run all
            # B-blocks before the correction blocks so wlo can arrive late
            x0_sb = xpool.tile([128, KT, 2, N_CHUNK], F8, name="x_sb",
                               tag="x")
            nc.sync.dma_start(out=wq_sb[:, 0], in_=wq[:, 0])
            nc.sync.dma_start(out=x0_sb[:, 0:2], in_=xq[:, 0, 0:2])
            nc.sync.dma_start(out=wq_sb[:, 1], in_=wq[:, 1])
            nc.sync.dma_start(out=x0_sb[:, 2:6], in_=xq[:, 0, 2:6])
            nc.sync.dma_start(out=x0_sb[:, 6:10, 0:1],
                              in_=xq[:, 0, 6:10, 0:1])
            nc.sync.dma_start(out=x0_sb[:, 10:KT], in_=xq[:, 0, 10:KT])
            nc.sync.dma_start(out=wq_sb[:, 2:4], in_=wq[:, 2:4])
            nc.sync.dma_start(out=wlo_sb, in_=wlo[:, :, :, :])

            # ---- main loop; sub-chunks: 7 full 512s + 2 halves of 256 ----
            # (c, n_sl within x tile); chunk 7's x tile serves both halves
            pending = None  # (h_tiles, n0, width) awaiting layer 2
            x_sb = x0_sb
            full = slice(0, N_CHUNK)
            subchunks = [(c, full) for c in range(N_CHUNKS)]
            for c, n_sl in subchunks:
                if n_sl.start == 0 and c > 0:
                    x_sb = xpool.tile([128, KT, 2, N_CHUNK], F8, name="x_sb",
                                      tag="x")
                    splits = [(0, 3), (3, 6)] if c == 1 else [(0, 6)]
                    for (ka, kb) in splits:
                        nc.sync.dma_start(out=x_sb[:, ka:kb],
                                          in_=xq[:, c, ka:kb])
                    nc.sync.dma_start(out=x_sb[:, 6:10, 0:1],
                                      in_=xq[:, c, 6:10, 0:1])
                    nc.sync.dma_start(out=x_sb[:, 10:KT],
                                      in_=xq[:, c, 10:KT])
                width = n_sl.stop - n_sl.start
                h_tiles = []
                if c == 0:
                    # deferred group structure: 4 open psum groups.  j0/j1
                    # consume x pieces k-interleaved (two matmuls per slice
                    # matches the DMA delivery rate); j2/j3 run after their
                    # weights land, the wlo correction blocks last.
                    phs = [psum_h.tile([128, width], F32, name=f"ph_{j}",
                                       tag="ph") for j in range(J_TILES)]
                    for i in range(N_B_TILES):
                        for j in (0, 1):
                            b_tile(phs[j], j, i, x_sb, n_sl)
                    for j in (2, 3):
                        l1_b_block(x_sb, j, n_sl, phs[j])
                    h8 = hpool.tile([128, J_TILES, 2, width], F8,
                                    name="h8", tag="h8")
                    for j in range(J_TILES):
                        l1_t3_block(x_sb, j, n_sl, phs[j])
                        h_sb = hpool.tile([128, width], BF16, name=f"h_{j}",
                                          tag=f"h_{j}")
                        nc.scalar.activation(
                            h_sb, phs[j], mybir.ActivationFunctionType.Relu,
                            bias=b1_sb[:, j:j + 1], scale=SH,
                        )
                        nc.vector.tensor_copy(h8[:, j, 0], h_sb)
                        nc.vector.tensor_sub(h8[:, j, 1], h_sb, h8[:, j, 0])
                        h_tiles.append(h_sb)
                else:
                    last = (c == N_CHUNKS - 1)
                    if not last:
                        h8 = hpool.tile([128, J_TILES, 2, width], F8,
                                        name="h8", tag="h8")
                    for j in range(J_TILES):
                        ph = psum_h.tile([128, width], F32, name=f"ph_{j}",
                                         tag="ph")
                        l1_b_block(x_sb, j, n_sl, ph)
                        l1_t3_block(x_sb, j, n_sl, ph)
                        if j == 1 and pending is not None:
                            l2_block(*pending)
                            pending = None
                        h_sb = hpool.tile([128, width], BF16, name=f"h_{j}",
                                          tag=f"h_{j}")
                        nc.scalar.activation(
                            h_sb, ph, mybir.ActivationFunctionType.Relu,
                            bias=b1_sb[:, j:j + 1], scale=SH,
                        )
                        if not last:
                            nc.vector.tensor_copy(h8[:, j, 0], h_sb)
                            nc.vector.tensor_sub(h8[:, j, 1], h_sb,
                                                 h8[:, j, 0])
                        h_tiles.append(h_sb)
                hx = h_tiles if c == N_CHUNKS - 1 else h8
                pending = (hx, c * N_CHUNK + n_sl.start, width)
            l2_block(*pending, final=True)

    nc.finalize()
    return nc


def _get_nc():
    if "nc" not in _NC_CACHE:
        _NC_CACHE["nc"] = _build_bass()
    return _NC_CACHE["nc"]


def _prep_in_maps(f, g, W1t, b1t, W1p, b1p, W2, b2):
    f2 = np.asarray(f, np.float32).reshape(BATCH, TRANS_H)
    g2 = np.asarray(g, np.float32).reshape(BATCH, PRED_H)

    # ---- weights: fp8 hi/lo split at scale SW, k-slices permuted ----
    ws = np.zeros((K_PAD, JOINT_H), np.float32)
    ws[:TRANS_H] = np.asarray(W1t, np.float32).T * SW
    ws[TRANS_H:K_TOTAL] = np.asarray(W1p, np.float32).T * SW
    whi = ws.astype(E4)
    wlo8 = (ws - whi.astype(np.float32)).astype(E4)

    # wq[p, jb, kk, jc] = whi[PERM[kk]*128+p, jb*128+jc]
    whi_s = whi.reshape(KT, 128, J_TILES, 128)[PERM]      # [kk, p, jb, jc]
    wq = np.ascontiguousarray(whi_s.transpose(1, 2, 0, 3))
    # correction tiles cover permuted slice positions (0,1),(2,3),(4,5)
    wlo_s = wlo8.reshape(KT, 128, JOINT_H)[PERM[:2 * T3]]
    wlo_t = np.ascontiguousarray(
        wlo_s.reshape(T3, 2, 128, JOINT_H).transpose(2, 0, 1, 3)
    )

    b1 = (np.asarray(b1t, np.float32) + np.asarray(b1p, np.float32)) * 32.0
    b1s = np.ascontiguousarray(b1.reshape(J_TILES, 128).T)
    w2t = np.asarray(W2, np.float32).T                    # [512, 29]
    w2b = np.ascontiguousarray(
        (w2t / 32.0).reshape(J_TILES, 128, NUM_LABELS).transpose(1, 0, 2)
    ).astype(BF)
    b2c = np.asarray(b2, np.float32).reshape(NUM_LABELS, 1)
    w2src = np.zeros((JOINT_H, LAB_PAD), np.float32)
    w2src[:, :NUM_LABELS] = w2t * S2
    w2hi = w2src.astype(E4)
    w2lo = (w2src - w2hi.astype(np.float32)).astype(E4)
    w2q = np.empty((128, J_TILES, 2, LAB_PAD), E4)
    w2q[:, :, 0] = w2hi.reshape(J_TILES, 128, LAB_PAD).transpose(1, 0, 2)
    w2q[:, :, 1] = w2q[:, :, 0]
    w2l = np.ascontiguousarray(
        w2lo.reshape(2, 2, 128, LAB_PAD).transpose(2, 0, 1, 3)
    )

    # ---- activations: fp8 hi/lo split at scale SX ----
    in_maps = []
    for core in range(N_CORES):
        sl = slice(core * N_PER_CORE, (core + 1) * N_PER_CORE)
        xs = np.zeros((N_PER_CORE, K_PAD), np.float32)
        xs[:, :TRANS_H] = f2[sl] * SX
        xs[:, TRANS_H:K_TOTAL] = g2[sl] * SX
        xhi = xs.astype(E4)
        xlo = (xs - xhi.astype(np.float32)).astype(E4)
        # [n, k] -> [p, c, kk, slot, n']  with row n = c*512+n', col= k*128+p
        xq = np.empty((128, N_CHUNKS, KT, 2, N_CHUNK), E4)
        xq[:, :, :, 0] = (xhi.reshape(N_CHUNKS, N_CHUNK, KT, 128)[:, :, PERM]
                          .transpose(3, 0, 2, 1))
        xq[:, :, :, 1] = (xlo.reshape(N_CHUNKS, N_CHUNK, KT, 128)[:, :, PERM]
                          .transpose(3, 0, 2, 1))
        in_maps.append({"xq": xq, "wq": wq, "wlo": wlo_t, "b1s": b1s,
                        "w2b": w2b, "b2c": b2c, "w2q": w2q, "w2l": w2l})
    return in_maps


def _gather(results):
    y = np.empty((1, BATCH, NUM_LABELS), np.float32)
    for core, r in enumerate(results):
        y[0, core * N_PER_CORE:(core + 1) * N_PER_CORE] = r["yT"].T
    return y


def _run(inputs, trace=False):
    in_maps = _prep_in_maps(
        inputs["f"], inputs["g"], inputs["W1t"], inputs["b1t"],
        inputs["W1p"], inputs["b1p"], inputs["W2"], inputs["b2"],
    )
    res = run_bass_kernel_spmd(
        _get_nc(), in_maps, core_ids=list(range(N_CORES)), trace=trace
    )
    return _gather(res.results), res


def kernel(**inputs) -> np.ndarray:
    out, _ = _run(inputs, trace=False)
    return out
